# revision 4
# baseline (speedup 1.0000x reference)
"""Trainium2 Bass kernel for nn_HamiltonianDynamics.

Math: with q = state[:, :8], p = state[:, 8:], every MLP evaluation in the
reference operates on per-batch means of q/p. Adding a constant c to every
element of a [8,256,256] block shifts its mean by exactly c, so the whole
leapfrog chain (g1, g2, g3), the casimir correction and the global norm are
computable from just per-batch sums and sums of squares:

  out = (state + off[b, half]) * scale
  off_q[b] = dt*g2[b,1]/Nq,  off_p[b] = -0.5*dt*(g1[b,0]+g3[b,0])/Nq
  norm^2   = sum_b,h ( ssq[b,h] + 2*off[b,h]*sum[b,h] + Nq*off[b,h]^2 )
  scale    = 1 - 0.1*err/(norm+1e-10)

Fully data-parallel SPMD: each core owns 4 whole batches, so the offsets
(the only per-element-visible quantity) are exactly computable locally.
Only `scale` couples cores — and scale-1 is O(err/norm) ~ 1e-13, i.e. ten
orders of magnitude below bf16 output resolution — so it is computed from
per-core unbiased estimates (local err mean; norm^2 from local sums plus a
2-tile sum-of-squares subsample), eliminating the collective entirely.

I/O is staged in bf16 (host converts): quantization contributes ~2e-3
norm-relative error vs the 2e-2 gate while halving HBM traffic. Stats are
accumulated in fp32 on-device; the elementwise transform computes in fp32
with bf16 in/out. bf16 (not fp16) keeps full relative precision on tiny
elements (wide exponent, no subnormal loss above 1e-38).

Engine-AP constraint: compute-engine APs must start at partition 0 (quarter
boundaries), so all per-batch row vectors live in separate [1,nb] tiles and
the 2-feature input layers are done as two accumulated K=1 matmuls.
"""

import numpy as np
from ml_dtypes import bfloat16

NCORES = 8
B, CH, H, W = 32, 16, 256, 256
BPC = B // NCORES          # batches per core
NTILES = BPC * 2           # (batch, half) tiles per core
P = 128
FREE = (CH // 2) * H * W // P   # 4096
NQ = (CH // 2) * H * W          # 524288
NSSQ = 2                   # tiles subsampled for the norm estimate

# packed-weights column layout (partitions x columns, f32)
_COLS = {}


def _col_layout():
    c = 0
    def put(name, cols):
        nonlocal c
        _COLS[name] = (c, c + cols)
        c += cols
    put("w1a", 128); put("w1b", 128); put("b1", 1)
    put("w2", 128); put("b2", 1)
    put("w3", 64); put("b3", 1)
    put("w4", 1); put("w4n", 1)
    put("w1t", 2); put("w2t", 128); put("w3t", 128)
    put("cw1a", 64); put("cw1b", 64); put("cb1", 1)
    put("cw2", 32); put("cb2", 1)
    put("cw3", 4)
    put("aux", 2)
    return c


NW = _col_layout()

_CACHE: dict = {}


def build_nc(ncores=NCORES, bpc=BPC, free=FREE):
    import concourse.bass as bass
    import concourse.bacc as bacc
    import concourse.tile as tile
    import concourse.mybir as mybir
    from contextlib import ExitStack

    f32 = mybir.dt.float32
    f16 = mybir.dt.bfloat16
    AL = mybir.AluOpType
    AF = mybir.ActivationFunctionType
    AX = mybir.AxisListType

    ntiles = bpc * 2
    nb = bpc
    nq = float(P * free)

    nc = bacc.Bacc("TRN2", target_bir_lowering=False, debug=False,
                   num_devices=ncores)

    x = nc.dram_tensor("x", [ntiles, P, free], f16, kind="ExternalInput").ap()
    w = nc.dram_tensor("w", [P, NW], f32, kind="ExternalInput").ap()
    y = nc.dram_tensor("y", [ntiles, P, free], f16, kind="ExternalOutput").ap()

    with tile.TileContext(nc) as tc, ExitStack() as ctx:
        xpool = ctx.enter_context(tc.tile_pool(name="xp", bufs=1))
        wpool = ctx.enter_context(tc.tile_pool(name="wp", bufs=1))
        scr = ctx.enter_context(tc.tile_pool(name="scr", bufs=2))
        ch = ctx.enter_context(tc.tile_pool(name="ch", bufs=2))
        keep = ctx.enter_context(tc.tile_pool(name="keep", bufs=1))
        psum = ctx.enter_context(tc.tile_pool(name="ps", bufs=4, space="PSUM"))

        ones_col = wpool.tile([128, 1], f32)     # lhsT for partition sums
        nc.vector.memset(ones_col[:], 1.0)
        ones_bc = wpool.tile([1, 128], f32)      # lhsT for partition broadcast
        nc.vector.memset(ones_bc[:], 1.0)

        # ---- phase A: load shard + per-(batch,half) stats ----
        # sums via DVE tensor_scalar identity with accum_out (runs in bf16
        # fast mode); sum-of-squares only on the first NSSQ tiles via ACT
        # Square+accum (norm estimate input).
        part_ps = psum.tile([1, ntiles + NSSQ], f32, tag="stat")
        xts, sts = [], []
        for t in range(ntiles):
            xt = xpool.tile([P, free], f16, tag=f"x{t}")
            nc.sync.dma_start(xt[:], x[t])
            st = keep.tile([128, 2 if t < NSSQ else 1], f32, tag=f"st{t}")
            nc.vector.tensor_scalar(xt[:], xt[:], scalar1=1.0, scalar2=0.0,
                                    op0=AL.mult, op1=AL.add,
                                    accum_out=st[:, 0:1])
            if t < NSSQ:
                sq = scr.tile([P, free], f16, tag=f"sq{t}")
                nc.scalar.activation(sq[:], xt[:], AF.Square,
                                     accum_out=st[:, 1:2])
            nc.tensor.matmul(part_ps[0:1, t:t + 1], ones_col[:], st[:, 0:1],
                             start=True, stop=True)
            if t < NSSQ:
                nc.tensor.matmul(part_ps[0:1, ntiles + t:ntiles + t + 1],
                                 ones_col[:], st[:, 1:2],
                                 start=True, stop=True)
            xts.append(xt)
            sts.append(st)

        # packed weights (single DMA; queued behind the shard loads)
        wt = wpool.tile([P, NW], f32)
        nc.sync.dma_start(wt[:], w)

        def wap(name):
            c0, c1 = _COLS[name]
            rows = {"w1a": 1, "w1b": 1, "cw1a": 1, "cw1b": 1,
                    "b3": 64, "w4": 64, "w4n": 64, "w3t": 64,
                    "cb1": 64, "cw2": 64, "cb2": 32, "cw3": 32,
                    "aux": 1}.get(name, 128)
            return wt[0:rows, c0:c1]

        # stats row [1, ntiles+NSSQ]: col t = sum of tile t, col ntiles+j =
        # ssq of tile j (j < NSSQ)
        r = keep.tile([1, ntiles + NSSQ], f32)
        nc.vector.tensor_copy(r[:], part_ps[:])
        mq = keep.tile([1, nb], f32)
        nc.vector.tensor_scalar(mq[:], r[0:1, 0:ntiles:2], scalar1=1.0 / nq,
                                scalar2=None, op0=AL.mult)
        mp = keep.tile([1, nb], f32)
        nc.vector.tensor_scalar(mp[:], r[0:1, 1:ntiles:2], scalar1=1.0 / nq,
                                scalar2=None, op0=AL.mult)

        # ---- phase C: scalar chain (features on partitions, batch on free) --
        def gH(mq_, mp_, want):
            """grad of sum(ham MLP) wrt (mq, mp): [1,nb] psum, row `want`."""
            p1 = psum.tile([128, nb], f32, tag="ps")
            nc.tensor.matmul(p1[:], wap("w1a"), mq_[:], start=True, stop=False)
            nc.tensor.matmul(p1[:], wap("w1b"), mp_[:], start=False, stop=True)
            h1 = ch.tile([128, nb], f32, tag="h1")
            nc.scalar.activation(h1[:], p1[:], AF.Tanh, bias=wap("b1"))
            p2 = psum.tile([128, nb], f32, tag="ps")
            nc.tensor.matmul(p2[:], wap("w2"), h1[:], start=True, stop=True)
            h2 = ch.tile([128, nb], f32, tag="h2")
            nc.scalar.activation(h2[:], p2[:], AF.Tanh, bias=wap("b2"))
            p3 = psum.tile([64, nb], f32, tag="ps")
            nc.tensor.matmul(p3[:], wap("w3"), h2[:], start=True, stop=True)
            h3 = ch.tile([64, nb], f32, tag="h3")
            nc.scalar.activation(h3[:], p3[:], AF.Tanh, bias=wap("b3"))
            # d3 = (1 - h3^2) * W4  ==  (h3^2) * (-W4) + W4
            d3 = ch.tile([64, nb], f32, tag="d3")
            nc.vector.tensor_tensor(d3[:], h3[:], h3[:], op=AL.mult)
            nc.vector.tensor_scalar(d3[:], d3[:], scalar1=wap("w4n"),
                                    scalar2=wap("w4"), op0=AL.mult, op1=AL.add)
            pd2 = psum.tile([128, nb], f32, tag="ps")
            nc.tensor.matmul(pd2[:], wap("w3t"), d3[:], start=True, stop=True)
            t2 = ch.tile([128, nb], f32, tag="t2")
            nc.vector.tensor_tensor(t2[:], h2[:], h2[:], op=AL.mult)
            nc.vector.tensor_scalar(t2[:], t2[:], scalar1=-1.0, scalar2=1.0,
                                    op0=AL.mult, op1=AL.add)
            d2 = ch.tile([128, nb], f32, tag="d2")
            nc.vector.tensor_tensor(d2[:], t2[:], pd2[:], op=AL.mult)
            pd1 = psum.tile([128, nb], f32, tag="ps")
            nc.tensor.matmul(pd1[:], wap("w2t"), d2[:], start=True, stop=True)
            t1 = ch.tile([128, nb], f32, tag="t1")
            nc.vector.tensor_tensor(t1[:], h1[:], h1[:], op=AL.mult)
            nc.vector.tensor_scalar(t1[:], t1[:], scalar1=-1.0, scalar2=1.0,
                                    op0=AL.mult, op1=AL.add)
            d1 = ch.tile([128, nb], f32, tag="d1")
            nc.vector.tensor_tensor(d1[:], t1[:], pd1[:], op=AL.mult)
            pg = psum.tile([1, nb], f32, tag="ps")
            col = 0 if want == "q" else 1
            w1t = wap("w1t")
            nc.tensor.matmul(pg[:], w1t[:, col:col + 1], d1[:],
                             start=True, stop=True)
            return pg

        def cas_h2(mq_, mp_, tag):
            """second hidden layer of casimir MLP -> [32,nb] sbuf."""
            q1 = psum.tile([64, nb], f32, tag="ps")
            nc.tensor.matmul(q1[:], wap("cw1a"), mq_[:], start=True, stop=False)
            nc.tensor.matmul(q1[:], wap("cw1b"), mp_[:], start=False, stop=True)
            g1 = ch.tile([64, nb], f32, tag="cg1")
            nc.scalar.activation(g1[:], q1[:], AF.Tanh, bias=wap("cb1"))
            q2 = psum.tile([32, nb], f32, tag="ps")
            nc.tensor.matmul(q2[:], wap("cw2"), g1[:], start=True, stop=True)
            g2 = ch.tile([32, nb], f32, tag=tag)
            nc.scalar.activation(g2[:], q2[:], AF.Tanh, bias=wap("cb2"))
            return g2

        aux = wap("aux")
        aux0, aux1 = aux[0:1, 0:1], aux[0:1, 1:2]

        pg1 = gH(mq, mp, "q")
        o1 = keep.tile([1, nb], f32)
        nc.vector.tensor_scalar(o1[:], pg1[:], scalar1=aux0, scalar2=None,
                                op0=AL.mult)
        mp2 = keep.tile([1, nb], f32)
        nc.vector.tensor_tensor(mp2[:], mp[:], o1[:], op=AL.add)
        pg2 = gH(mq, mp2, "p")
        offq = keep.tile([1, nb], f32)
        nc.vector.tensor_scalar(offq[:], pg2[:], scalar1=aux1, scalar2=None,
                                op0=AL.mult)
        mq3 = keep.tile([1, nb], f32)
        nc.vector.tensor_tensor(mq3[:], mq[:], offq[:], op=AL.add)
        pg3 = gH(mq3, mp2, "q")
        o3 = keep.tile([1, nb], f32)
        nc.vector.tensor_scalar(o3[:], pg3[:], scalar1=aux0, scalar2=None,
                                op0=AL.mult)
        offp = keep.tile([1, nb], f32)
        nc.vector.tensor_tensor(offp[:], o1[:], o3[:], op=AL.add)
        mpn = keep.tile([1, nb], f32)
        nc.vector.tensor_tensor(mpn[:], mp[:], offp[:], op=AL.add)

        # casimir err estimate: mean over the core's own batches
        g2o = cas_h2(mq, mp, "g2o")
        g2n = cas_h2(mq3, mpn, "g2n")
        dh = ch.tile([32, nb], f32, tag="dh")
        nc.vector.tensor_tensor(dh[:], g2n[:], g2o[:], op=AL.subtract)
        qd = psum.tile([4, nb], f32, tag="ps")
        nc.tensor.matmul(qd[:], wap("cw3"), dh[:], start=True, stop=True)
        dsum = keep.tile([4, 1], f32)
        nc.vector.tensor_reduce(dsum[:], qd[:], axis=AX.X, op=AL.add)
        pe = psum.tile([1, 1], f32, tag="ps")
        nc.tensor.matmul(pe[:], ones_col[0:4, 0:1], dsum[:], start=True,
                         stop=True)
        err = keep.tile([1, 1], f32)
        nc.vector.tensor_copy(err[:], pe[:])

        # norm^2 estimate: exact per-batch correction terms (x8 to global),
        # raw ssq from the NSSQ-tile subsample (x ntiles*ncores/NSSQ)
        n2 = keep.tile([1, nb], f32)
        u1 = ch.tile([1, nb], f32, tag="u1")
        nc.vector.tensor_tensor(u1[:], offq[:], r[0:1, 0:ntiles:2], op=AL.mult)
        nc.vector.tensor_scalar(u1[:], u1[:], scalar1=2.0, scalar2=None,
                                op0=AL.mult)
        u2 = ch.tile([1, nb], f32, tag="u2")
        nc.vector.tensor_tensor(u2[:], offq[:], offq[:], op=AL.mult)
        nc.vector.tensor_scalar(u2[:], u2[:], scalar1=nq, scalar2=None,
                                op0=AL.mult)
        nc.vector.tensor_tensor(n2[:], u1[:], u2[:], op=AL.add)
        v1 = ch.tile([1, nb], f32, tag="v1")
        nc.vector.tensor_tensor(v1[:], offp[:], r[0:1, 1:ntiles:2], op=AL.mult)
        nc.vector.tensor_scalar(v1[:], v1[:], scalar1=2.0, scalar2=None,
                                op0=AL.mult)
        v2 = ch.tile([1, nb], f32, tag="v2")
        nc.vector.tensor_tensor(v2[:], offp[:], offp[:], op=AL.mult)
        nc.vector.tensor_scalar(v2[:], v2[:], scalar1=nq, scalar2=None,
                                op0=AL.mult)
        nc.vector.tensor_tensor(n2[:], n2[:], v1[:], op=AL.add)
        nc.vector.tensor_tensor(n2[:], n2[:], v2[:], op=AL.add)
        nsum = keep.tile([1, 1], f32)
        nc.vector.tensor_reduce(nsum[:], n2[:], axis=AX.X, op=AL.add)
        ssq2 = keep.tile([1, 1], f32)
        nc.vector.tensor_tensor(ssq2[:], r[0:1, ntiles:ntiles + 1],
                                r[0:1, ntiles + 1:ntiles + 2], op=AL.add)
        # norm2 = (ncores*ntiles/NSSQ)*ssq2 + ncores*nsum
        nc.vector.tensor_scalar(nsum[:], nsum[:], scalar1=float(ncores),
                                scalar2=None, op0=AL.mult)
        nc.vector.tensor_scalar(ssq2[:], ssq2[:],
                                scalar1=float(ncores * ntiles) / NSSQ,
                                scalar2=None, op0=AL.mult)
        norm2 = keep.tile([1, 1], f32)
        nc.vector.tensor_tensor(norm2[:], ssq2[:], nsum[:], op=AL.add)
        nrm = keep.tile([1, 1], f32)
        nc.scalar.sqrt(nrm[:], norm2[:])
        den = keep.tile([1, 1], f32)
        nc.vector.tensor_scalar(den[:], nrm[:], scalar1=1e-10, scalar2=None,
                                op0=AL.add)
        rec = keep.tile([1, 1], f32)
        nc.vector.reciprocal(rec[:], den[:])
        scv = keep.tile([1, 1], f32)
        nc.vector.tensor_tensor(scv[:], err[:], rec[:], op=AL.mult)
        # scale = 1 - (0.1/(4*nb)) * errsum / (norm+1e-10)
        nc.vector.tensor_scalar(scv[:], scv[:], scalar1=-0.1 / (4.0 * nb),
                                scalar2=1.0, op0=AL.mult, op1=AL.add)

        # ---- phase D: scale offsets + partition broadcast ----
        Bv = keep.tile([1, 2 * nb + 1], f32)
        nc.vector.tensor_scalar(Bv[0:1, 0:nb], offq[:], scalar1=scv[0:1, 0:1],
                                scalar2=None, op0=AL.mult)
        nc.vector.tensor_scalar(Bv[0:1, nb:2 * nb], offp[:],
                                scalar1=scv[0:1, 0:1], scalar2=None,
                                op0=AL.mult)
        nc.vector.tensor_copy(Bv[0:1, 2 * nb:2 * nb + 1], scv[:])
        poffb = psum.tile([128, 2 * nb + 1], f32, tag="ps")
        nc.tensor.matmul(poffb[:], ones_bc[:], Bv[:], start=True, stop=True)
        offb = keep.tile([128, 2 * nb + 1], f32)
        nc.vector.tensor_copy(offb[:], poffb[:])

        # ---- phase E: in-place transform + store ----
        for t in range(ntiles):
            bl, h = t // 2, t % 2
            col = h * nb + bl
            xt = xts[t]
            nc.vector.tensor_scalar(xt[:], xt[:],
                                    scalar1=offb[:, 2 * nb:2 * nb + 1],
                                    scalar2=offb[:, col:col + 1],
                                    op0=AL.mult, op1=AL.add)
            nc.sync.dma_start(y[t], xt[:])

    nc.compile()
    return nc


def make_in_maps(inputs, ncores=NCORES, bpc=BPC, free=FREE):
    state = np.asarray(inputs["state"])
    dt = float(np.asarray(inputs["dt"]))
    nq = float(P * free)
    f = np.float32
    g = lambda k: np.ascontiguousarray(np.asarray(inputs[k], dtype=f))
    hW1, hW2, hW3, hW4 = g("hW1"), g("hW2"), g("hW3"), g("hW4")
    cW1 = g("cW1")

    wpack = np.zeros((P, NW), dtype=f)
    def put(name, arr):
        c0, c1 = _COLS[name]
        arr = np.asarray(arr, dtype=f)
        wpack[:arr.shape[0], c0:c1] = arr
    # w1a/w1b/cw1a/cw1b are [1,n] row tiles living on partition 0
    wpack[0, _COLS["w1a"][0]:_COLS["w1a"][1]] = hW1[0, :]
    wpack[0, _COLS["w1b"][0]:_COLS["w1b"][1]] = hW1[1, :]
    put("b1", g("hb1").reshape(128, 1))
    put("w2", hW2)
    put("b2", g("hb2").reshape(128, 1))
    put("w3", hW3)
    put("b3", g("hb3").reshape(64, 1))
    put("w4", hW4.reshape(64, 1))
    put("w4n", -hW4.reshape(64, 1))
    put("w1t", hW1.T)
    put("w2t", hW2.T)
    put("w3t", hW3.T)
    wpack[0, _COLS["cw1a"][0]:_COLS["cw1a"][1]] = cW1[0, :]
    wpack[0, _COLS["cw1b"][0]:_COLS["cw1b"][1]] = cW1[1, :]
    put("cb1", g("cb1").reshape(64, 1))
    put("cw2", g("cW2"))
    put("cb2", g("cb2").reshape(32, 1))
    put("cw3", g("cW3"))
    wpack[0, _COLS["aux"][0]] = -0.5 * dt / nq
    wpack[0, _COLS["aux"][0] + 1] = dt / nq

    in_maps = []
    for i in range(ncores):
        shard = state[i * bpc:(i + 1) * bpc].astype(bfloat16).reshape(
            2 * bpc, P, free)
        in_maps.append({"x": shard, "w": wpack})
    return in_maps


def kernel(**inputs):
    from concourse.bass_utils import run_bass_kernel_spmd

    if "nc" not in _CACHE:
        _CACHE["nc"] = build_nc()
    nc = _CACHE["nc"]
    in_maps = make_in_maps(inputs)
    res = run_bass_kernel_spmd(nc, in_maps, list(range(NCORES)))
    out = np.concatenate(
        [res.results[i]["y"].astype(np.float32).reshape(BPC, CH, H, W)
         for i in range(NCORES)],
        axis=0)
    return out


# revision 12
# speedup vs baseline: 1.1326x; 1.1326x over previous
"""Trainium2 Bass kernel for nn_HamiltonianDynamics.

Math: with q = state[:, :8], p = state[:, 8:], every MLP evaluation in the
reference operates on per-batch means of q/p. Adding a constant c to every
element of a [8,256,256] block shifts its mean by exactly c, so the whole
leapfrog chain (g1, g2, g3), the casimir correction and the global norm are
computable from just per-batch sums and sums of squares:

  out = (state + off[b, half]) * scale
  off_q[b] = dt*g2[b,1]/Nq,  off_p[b] = -0.5*dt*(g1[b,0]+g3[b,0])/Nq
  norm^2   = sum_b,h ( ssq[b,h] + 2*off[b,h]*sum[b,h] + Nq*off[b,h]^2 )
  scale    = 1 - 0.1*err/(norm+1e-10)

Fully data-parallel SPMD: each core owns 4 whole batches, so the offsets
(the only per-element-visible quantity) are exactly computable locally.
Only `scale` couples cores — and scale-1 is O(err/norm) ~ 1e-13, i.e. ten
orders of magnitude below bf16 output resolution — so it is computed from
per-core unbiased estimates (local err mean; norm^2 from local sums plus a
2-tile sum-of-squares subsample), eliminating the collective entirely.

I/O is staged in bf16 (host converts): quantization contributes ~2e-3
norm-relative error vs the 2e-2 gate while halving HBM traffic. Stats are
accumulated in fp32 on-device; the elementwise transform computes in fp32
with bf16 in/out. bf16 (not fp16) keeps full relative precision on tiny
elements (wide exponent, no subnormal loss above 1e-38).

Engine-AP constraint: compute-engine APs must start at partition 0 (quarter
boundaries), so all per-batch row vectors live in separate [1,nb] tiles and
the 2-feature input layers are done as two accumulated K=1 matmuls.
"""

import numpy as np
from ml_dtypes import bfloat16

NCORES = 8
B, CH, H, W = 32, 16, 256, 256
BPC = B // NCORES          # batches per core
NTILES = BPC * 2           # (batch, half) tiles per core
P = 128
FREE = (CH // 2) * H * W // P   # 4096
NQ = (CH // 2) * H * W          # 524288
NSSQ = 2                   # tiles subsampled for the norm estimate

# packed-weights column layout (partitions x columns, f32)
_COLS = {}


def _col_layout():
    c = 0
    def put(name, cols):
        nonlocal c
        _COLS[name] = (c, c + cols)
        c += cols
    put("w1a", 128); put("w1b", 128); put("b1", 1)
    put("w2", 128); put("b2", 1)
    put("w3", 64); put("b3", 1)
    put("w4", 1); put("w4n", 1)
    put("w1t", 2); put("w2t", 128); put("w3t", 128)
    put("cw1a", 64); put("cw1b", 64); put("cb1", 1)
    put("cw2", 32); put("cb2", 1)
    put("cw3", 4); put("werr", 1)
    put("aux", 3)
    return c


NW = _col_layout()

_CACHE: dict = {}


def build_nc(ncores=NCORES, bpc=BPC, free=FREE):
    import concourse.bass as bass
    import concourse.bacc as bacc
    import concourse.tile as tile
    import concourse.mybir as mybir
    from contextlib import ExitStack

    f32 = mybir.dt.float32
    f16 = mybir.dt.bfloat16
    AL = mybir.AluOpType
    AF = mybir.ActivationFunctionType
    AX = mybir.AxisListType

    ntiles = bpc * 2
    nb = bpc
    nq = float(P * free)

    nc = bacc.Bacc("TRN2", target_bir_lowering=False, debug=False,
                   num_devices=ncores)

    x = nc.dram_tensor("x", [ntiles, P, free], f16, kind="ExternalInput").ap()
    w = nc.dram_tensor("w", [P, NW], f32, kind="ExternalInput").ap()
    y = nc.dram_tensor("y", [ntiles, P, free], f16, kind="ExternalOutput").ap()

    with tile.TileContext(nc) as tc, ExitStack() as ctx:
        xpool = ctx.enter_context(tc.tile_pool(name="xp", bufs=1))
        wpool = ctx.enter_context(tc.tile_pool(name="wp", bufs=1))
        scr = ctx.enter_context(tc.tile_pool(name="scr", bufs=2))
        ch = ctx.enter_context(tc.tile_pool(name="ch", bufs=2))
        keep = ctx.enter_context(tc.tile_pool(name="keep", bufs=1))
        psum = ctx.enter_context(tc.tile_pool(name="ps", bufs=4, space="PSUM"))

        ones_col = wpool.tile([128, 1], f32)     # lhsT for partition sums
        nc.vector.memset(ones_col[:], 1.0)
        ones_bc = wpool.tile([1, 128], f32)      # lhsT for partition broadcast
        nc.vector.memset(ones_bc[:], 1.0)

        # ---- phase A: load shard + per-(batch,half) stats ----
        # Each tile loads as two half-chunks so the DVE sum accumulation
        # (tensor_scalar identity with accum_out, bf16 fast mode) trails the
        # DMA stream by only half a tile. The two halves' partition sums are
        # folded in PSUM via accumulated ones-matmuls. Sum-of-squares only on
        # the first NSSQ tiles via ACT Square+accum (norm estimate input).
        hf = free // 2
        part_ps = psum.tile([1, ntiles + NSSQ], f32, tag="stat")
        xts = []
        for t in range(ntiles):
            xt = xpool.tile([P, free], f16, tag=f"x{t}")
            st = keep.tile([128, 2], f32, tag=f"st{t}")
            for c in range(2):
                sl = slice(c * hf, (c + 1) * hf)
                nc.sync.dma_start(xt[:, sl], x[t][:, sl])
                nc.vector.tensor_scalar(xt[:, sl], xt[:, sl], scalar1=1.0,
                                        scalar2=0.0, op0=AL.mult, op1=AL.add,
                                        accum_out=st[:, c:c + 1])
                nc.tensor.matmul(part_ps[0:1, t:t + 1], ones_col[:],
                                 st[:, c:c + 1], start=(c == 0), stop=(c == 1))
            if t < NSSQ:
                st2 = keep.tile([128, 1], f32, tag=f"ss{t}")
                sq = scr.tile([P, free], f16, tag=f"sq{t}")
                nc.scalar.activation(sq[:], xt[:], AF.Square,
                                     accum_out=st2[:, 0:1])
                nc.tensor.matmul(part_ps[0:1, ntiles + t:ntiles + t + 1],
                                 ones_col[:], st2[:, 0:1],
                                 start=True, stop=True)
            xts.append(xt)

        # packed weights (single DMA; queued behind the shard loads)
        wt = wpool.tile([P, NW], f32)
        nc.sync.dma_start(wt[:], w)

        def wap(name):
            c0, c1 = _COLS[name]
            rows = {"w1a": 1, "w1b": 1, "cw1a": 1, "cw1b": 1,
                    "b3": 64, "w4": 64, "w4n": 64, "w3t": 64,
                    "cb1": 64, "cw2": 64, "cb2": 32, "cw3": 32,
                    "werr": 32, "aux": 1}.get(name, 128)
            return wt[0:rows, c0:c1]

        # stats row [1, ntiles+NSSQ]: col t = sum of tile t, col ntiles+j =
        # ssq of tile j (j < NSSQ)
        r = keep.tile([1, ntiles + NSSQ], f32)
        nc.vector.tensor_copy(r[:], part_ps[:])
        mq = keep.tile([1, nb], f32)
        nc.vector.tensor_scalar(mq[:], r[0:1, 0:ntiles:2], scalar1=1.0 / nq,
                                scalar2=None, op0=AL.mult)
        mp = keep.tile([1, nb], f32)
        nc.vector.tensor_scalar(mp[:], r[0:1, 1:ntiles:2], scalar1=1.0 / nq,
                                scalar2=None, op0=AL.mult)

        # ---- phase C: scalar chain (features on partitions, batch on free) --
        def gH(mq_, mp_):
            """backprop of sum(ham MLP) wrt inputs: returns d1 [128,nb] sbuf
            (pre-W1 sensitivities); rows of the input grad come from
            w1t-column matmuls on it."""
            p1 = psum.tile([128, nb], f32, tag="ps")
            nc.tensor.matmul(p1[:], wap("w1a"), mq_[:], start=True, stop=False)
            nc.tensor.matmul(p1[:], wap("w1b"), mp_[:], start=False, stop=True)
            h1 = ch.tile([128, nb], f32, tag="h1")
            nc.scalar.activation(h1[:], p1[:], AF.Tanh, bias=wap("b1"))
            p2 = psum.tile([128, nb], f32, tag="ps")
            nc.tensor.matmul(p2[:], wap("w2"), h1[:], start=True, stop=True)
            h2 = ch.tile([128, nb], f32, tag="h2")
            nc.scalar.activation(h2[:], p2[:], AF.Tanh, bias=wap("b2"))
            p3 = psum.tile([64, nb], f32, tag="ps")
            nc.tensor.matmul(p3[:], wap("w3"), h2[:], start=True, stop=True)
            h3 = ch.tile([64, nb], f32, tag="h3")
            nc.scalar.activation(h3[:], p3[:], AF.Tanh, bias=wap("b3"))
            # d3 = (1 - h3^2) * W4  ==  (h3^2) * (-W4) + W4
            d3 = ch.tile([64, nb], f32, tag="d3")
            nc.vector.tensor_tensor(d3[:], h3[:], h3[:], op=AL.mult)
            nc.vector.tensor_scalar(d3[:], d3[:], scalar1=wap("w4n"),
                                    scalar2=wap("w4"), op0=AL.mult, op1=AL.add)
            pd2 = psum.tile([128, nb], f32, tag="ps")
            nc.tensor.matmul(pd2[:], wap("w3t"), d3[:], start=True, stop=True)
            t2 = ch.tile([128, nb], f32, tag="t2")
            nc.vector.tensor_tensor(t2[:], h2[:], h2[:], op=AL.mult)
            nc.vector.tensor_scalar(t2[:], t2[:], scalar1=-1.0, scalar2=1.0,
                                    op0=AL.mult, op1=AL.add)
            d2 = ch.tile([128, nb], f32, tag="d2")
            nc.vector.tensor_tensor(d2[:], t2[:], pd2[:], op=AL.mult)
            pd1 = psum.tile([128, nb], f32, tag="ps")
            nc.tensor.matmul(pd1[:], wap("w2t"), d2[:], start=True, stop=True)
            t1 = ch.tile([128, nb], f32, tag="t1")
            nc.vector.tensor_tensor(t1[:], h1[:], h1[:], op=AL.mult)
            nc.vector.tensor_scalar(t1[:], t1[:], scalar1=-1.0, scalar2=1.0,
                                    op0=AL.mult, op1=AL.add)
            d1 = ch.tile([128, nb], f32, tag="d1")
            nc.vector.tensor_tensor(d1[:], t1[:], pd1[:], op=AL.mult)
            return d1

        def cas_h2(mq_, mp_, tag):
            """second hidden layer of casimir MLP -> [32,nb] sbuf."""
            q1 = psum.tile([64, nb], f32, tag="ps")
            nc.tensor.matmul(q1[:], wap("cw1a"), mq_[:], start=True, stop=False)
            nc.tensor.matmul(q1[:], wap("cw1b"), mp_[:], start=False, stop=True)
            g1 = ch.tile([64, nb], f32, tag="cg1")
            nc.scalar.activation(g1[:], q1[:], AF.Tanh, bias=wap("cb1"))
            q2 = psum.tile([32, nb], f32, tag="ps")
            nc.tensor.matmul(q2[:], wap("cw2"), g1[:], start=True, stop=True)
            g2 = ch.tile([32, nb], f32, tag=tag)
            nc.scalar.activation(g2[:], q2[:], AF.Tanh, bias=wap("cb2"))
            return g2

        aux = wap("aux")
        aux1, aux2 = aux[0:1, 1:2], aux[0:1, 2:3]

        # casimir at the original means — emitted first so its PE/ACT ops
        # fill the gH pipeline handoff gaps
        g2o = cas_h2(mq, mp, "g2o")

        # The three leapfrog gradient evaluations sit within O(dt*g/Nq)
        # ~ 1e-7 of the same point, so g1 == g2 == g3 to ~1e-6 relative and
        # one backprop supplies both offset rows:
        #   offq = dt*g[p]/Nq, offp = -dt*g[q]/Nq
        # (the collapse changes the offsets by ~1e-13 absolute — seven
        # orders below the bf16 output ulp).
        d1 = gH(mq, mp)
        w1t = wap("w1t")
        pgq = psum.tile([1, nb], f32, tag="ps")
        nc.tensor.matmul(pgq[:], w1t[:, 0:1], d1[:], start=True, stop=True)
        pgp = psum.tile([1, nb], f32, tag="ps")
        nc.tensor.matmul(pgp[:], w1t[:, 1:2], d1[:], start=True, stop=True)
        offq = keep.tile([1, nb], f32)
        nc.vector.tensor_scalar(offq[:], pgp[:], scalar1=aux1, scalar2=None,
                                op0=AL.mult)
        offp = keep.tile([1, nb], f32)
        nc.vector.tensor_scalar(offp[:], pgq[:], scalar1=aux2, scalar2=None,
                                op0=AL.mult)
        mq3 = keep.tile([1, nb], f32)
        nc.vector.tensor_tensor(mq3[:], mq[:], offq[:], op=AL.add)
        mpn = keep.tile([1, nb], f32)
        nc.vector.tensor_tensor(mpn[:], mp[:], offp[:], op=AL.add)

        # casimir err estimate: mean over the core's own batches (PE/ACT ops;
        # the norm^2 DVE work below runs concurrently)
        g2n = cas_h2(mq3, mpn, "g2n")

        # norm^2 estimate: exact per-batch correction terms (x8 to global),
        # raw ssq from the NSSQ-tile subsample (x ntiles*ncores/NSSQ)
        n2 = keep.tile([1, nb], f32)
        u1 = ch.tile([1, nb], f32, tag="u1")
        nc.vector.tensor_tensor(u1[:], offq[:], r[0:1, 0:ntiles:2], op=AL.mult)
        nc.vector.tensor_scalar(u1[:], u1[:], scalar1=2.0, scalar2=None,
                                op0=AL.mult)
        u2 = ch.tile([1, nb], f32, tag="u2")
        nc.vector.tensor_tensor(u2[:], offq[:], offq[:], op=AL.mult)
        nc.vector.tensor_scalar(u2[:], u2[:], scalar1=nq, scalar2=None,
                                op0=AL.mult)
        nc.vector.tensor_tensor(n2[:], u1[:], u2[:], op=AL.add)
        v1 = ch.tile([1, nb], f32, tag="v1")
        nc.vector.tensor_tensor(v1[:], offp[:], r[0:1, 1:ntiles:2], op=AL.mult)
        nc.vector.tensor_scalar(v1[:], v1[:], scalar1=2.0, scalar2=None,
                                op0=AL.mult)
        v2 = ch.tile([1, nb], f32, tag="v2")
        nc.vector.tensor_tensor(v2[:], offp[:], offp[:], op=AL.mult)
        nc.vector.tensor_scalar(v2[:], v2[:], scalar1=nq, scalar2=None,
                                op0=AL.mult)
        nc.vector.tensor_tensor(n2[:], n2[:], v1[:], op=AL.add)
        nc.vector.tensor_tensor(n2[:], n2[:], v2[:], op=AL.add)
        nsum = keep.tile([1, 1], f32)
        nc.vector.tensor_reduce(nsum[:], n2[:], axis=AX.X, op=AL.add)
        ssq2 = keep.tile([1, 1], f32)
        nc.vector.tensor_tensor(ssq2[:], r[0:1, ntiles:ntiles + 1],
                                r[0:1, ntiles + 1:ntiles + 2], op=AL.add)
        # norm2 = (ncores*ntiles/NSSQ)*ssq2 + ncores*nsum
        nc.vector.tensor_scalar(nsum[:], nsum[:], scalar1=float(ncores),
                                scalar2=None, op0=AL.mult)
        nc.vector.tensor_scalar(ssq2[:], ssq2[:],
                                scalar1=float(ncores * ntiles) / NSSQ,
                                scalar2=None, op0=AL.mult)
        norm2 = keep.tile([1, 1], f32)
        nc.vector.tensor_tensor(norm2[:], ssq2[:], nsum[:], op=AL.add)
        nrm = keep.tile([1, 1], f32)
        nc.scalar.sqrt(nrm[:], norm2[:])
        den = keep.tile([1, 1], f32)
        nc.vector.tensor_scalar(den[:], nrm[:], scalar1=1e-10, scalar2=None,
                                op0=AL.add)
        rec = keep.tile([1, 1], f32)
        nc.vector.reciprocal(rec[:], den[:])
        # fold the -0.1/(4*nb) err-mean factor into the reciprocal while the
        # err path is still in flight
        recs = keep.tile([1, 1], f32)
        nc.vector.tensor_scalar(recs[:], rec[:], scalar1=-0.1 / (4.0 * nb),
                                scalar2=None, op0=AL.mult)

        # err tail: dh = g2n - g2o; errsum = sum(werr[j] * dh[j,b]) with
        # werr = cW3 @ ones4 folded on the host (kills two engine handoffs)
        dh = ch.tile([32, nb], f32, tag="dh")
        nc.vector.tensor_tensor(dh[:], g2n[:], g2o[:], op=AL.subtract)
        dws = keep.tile([32, 1], f32)
        dwt = ch.tile([32, nb], f32, tag="dwt")
        nc.vector.tensor_scalar(dwt[:], dh[:], scalar1=wap("werr"),
                                scalar2=0.0, op0=AL.mult, op1=AL.add,
                                accum_out=dws[:, 0:1])
        pe = psum.tile([1, 1], f32, tag="ps")
        nc.tensor.matmul(pe[:], ones_col[0:32, 0:1], dws[:], start=True,
                         stop=True)
        # scale = 1 - (0.1/(4*nb)) * errsum / (norm+1e-10)
        scv = keep.tile([1, 1], f32)
        nc.vector.tensor_scalar(scv[:], pe[:], scalar1=recs[0:1, 0:1],
                                scalar2=1.0, op0=AL.mult, op1=AL.add)

        # ---- phase D: scale offsets + partition broadcast ----
        Bv = keep.tile([1, 2 * nb + 1], f32)
        nc.vector.tensor_scalar(Bv[0:1, 0:nb], offq[:], scalar1=scv[0:1, 0:1],
                                scalar2=None, op0=AL.mult)
        nc.vector.tensor_scalar(Bv[0:1, nb:2 * nb], offp[:],
                                scalar1=scv[0:1, 0:1], scalar2=None,
                                op0=AL.mult)
        nc.vector.tensor_copy(Bv[0:1, 2 * nb:2 * nb + 1], scv[:])
        poffb = psum.tile([128, 2 * nb + 1], f32, tag="ps")
        nc.tensor.matmul(poffb[:], ones_bc[:], Bv[:], start=True, stop=True)
        offb = keep.tile([128, 2 * nb + 1], f32)
        nc.vector.tensor_copy(offb[:], poffb[:])

        # ---- phase E: in-place transform + store (half tiles so the first
        # store launches half a tile after scale lands) ----
        for t in range(ntiles):
            bl, h = t // 2, t % 2
            col = h * nb + bl
            xt = xts[t]
            for c in range(2):
                sl = slice(c * hf, (c + 1) * hf)
                nc.vector.tensor_scalar(xt[:, sl], xt[:, sl],
                                        scalar1=offb[:, 2 * nb:2 * nb + 1],
                                        scalar2=offb[:, col:col + 1],
                                        op0=AL.mult, op1=AL.add)
                nc.sync.dma_start(y[t][:, sl], xt[:, sl])

    nc.compile()
    return nc


def make_in_maps(inputs, ncores=NCORES, bpc=BPC, free=FREE):
    state = np.asarray(inputs["state"])
    dt = float(np.asarray(inputs["dt"]))
    nq = float(P * free)
    f = np.float32
    g = lambda k: np.ascontiguousarray(np.asarray(inputs[k], dtype=f))
    hW1, hW2, hW3, hW4 = g("hW1"), g("hW2"), g("hW3"), g("hW4")
    cW1 = g("cW1")

    wpack = np.zeros((P, NW), dtype=f)
    def put(name, arr):
        c0, c1 = _COLS[name]
        arr = np.asarray(arr, dtype=f)
        wpack[:arr.shape[0], c0:c1] = arr
    # w1a/w1b/cw1a/cw1b are [1,n] row tiles living on partition 0
    wpack[0, _COLS["w1a"][0]:_COLS["w1a"][1]] = hW1[0, :]
    wpack[0, _COLS["w1b"][0]:_COLS["w1b"][1]] = hW1[1, :]
    put("b1", g("hb1").reshape(128, 1))
    put("w2", hW2)
    put("b2", g("hb2").reshape(128, 1))
    put("w3", hW3)
    put("b3", g("hb3").reshape(64, 1))
    put("w4", hW4.reshape(64, 1))
    put("w4n", -hW4.reshape(64, 1))
    put("w1t", hW1.T)
    put("w2t", hW2.T)
    put("w3t", hW3.T)
    wpack[0, _COLS["cw1a"][0]:_COLS["cw1a"][1]] = cW1[0, :]
    wpack[0, _COLS["cw1b"][0]:_COLS["cw1b"][1]] = cW1[1, :]
    put("cb1", g("cb1").reshape(64, 1))
    put("cw2", g("cW2"))
    put("cb2", g("cb2").reshape(32, 1))
    put("cw3", g("cW3"))
    put("werr", g("cW3") @ np.ones((4, 1), dtype=f))
    wpack[0, _COLS["aux"][0]] = -0.5 * dt / nq
    wpack[0, _COLS["aux"][0] + 1] = dt / nq
    wpack[0, _COLS["aux"][0] + 2] = -dt / nq

    in_maps = []
    for i in range(ncores):
        shard = state[i * bpc:(i + 1) * bpc].astype(bfloat16).reshape(
            2 * bpc, P, free)
        in_maps.append({"x": shard, "w": wpack})
    return in_maps


def kernel(**inputs):
    from concourse.bass_utils import run_bass_kernel_spmd

    if "nc" not in _CACHE:
        _CACHE["nc"] = build_nc()
    nc = _CACHE["nc"]
    in_maps = make_in_maps(inputs)
    res = run_bass_kernel_spmd(nc, in_maps, list(range(NCORES)))
    out = np.concatenate(
        [res.results[i]["y"].astype(np.float32).reshape(BPC, CH, H, W)
         for i in range(NCORES)],
        axis=0)
    return out
